# revision 2
# baseline (speedup 1.0000x reference)
"""LocallyConnected2D (B=16, 32x32, CIN=COUT=64, 3x3, pad=1) on 8 TRN2 NeuronCores.

Shard the 32 output rows across 8 cores (4 rows each); weights are repacked on
the host into a per-core, DMA-friendly layout (128 partitions, fully
contiguous). Per output pixel: 5 PSUM-accumulating matmuls — 4 with K=128
(consecutive tap pairs 2p,2p+1 stacked on partition halves; the relative
column shift between the two taps is baked into the x tile placement) plus
one K=64 matmul for tap 8. M=16 (batch), N=64 (cout); 4 pixels run
concurrently in the PE array via column tile_position. Bias added on host.

out[b,i,j,o] = sum_{c,k} x_pad[b, i+di, j+dj, c] * W[o,c,i,j,k], k=3*di+dj.

Host layouts (per core c, local row r, i = 4c+r):
  w_pairs [4, 128, 8192]: [r, 64m+cin, o*128 + j*4 + p] = W[o, cin, i, j, 2p+m]
  w_sing  [4,  64, 2048]: [r, cin,     o*32  + j      ] = W[o, cin, i, j, 8]
  xt      [6,  64,  512]: [rin, cin, j*16+b] = x_pad[b, 4c+rin, j, cin]
  out     [4, 4, 16, 8, 64]: [r, jj, b, g, o] = out[b, i, 4g+jj, o]

x tiles per row (576 = 36 cols * 16 batch; base: col j' stored at (j'+2)*16):
  pair p taps (2p, 2p+1): dj = (p%3, (p+1)%3)-ish; concretely
    P0 taps(0,1) rows(r,r)     dj(0,1)  P1 taps(2,3) rows(r,r+1) dj(2,0)
    P2 taps(4,5) rows(r+1,r+1) dj(1,2)  P3 taps(6,7) rows(r+2,r+2) dj(0,1)
  half m=1 placement shift = (dj0 - dj1) cols; lhsT AP offset = (j+dj0+1)*16.
  Tap 8 (row r+2, dj=2) reads P3's tile, partitions 0-63, offset (j+3)*16.
"""

import numpy as np

B, IH, IW, CIN = 16, 32, 32, 64
COUT, OH, OW = 64, 32, 32
NCORES, RPC = 8, 4

# per pair: (row0, row1, dj0, dj1)
PAIRS = [(0, 0, 0, 1), (0, 1, 2, 0), (1, 1, 1, 2), (2, 2, 0, 1)]

_NC = None


def _build_nc():
    import concourse.bacc as bacc
    import concourse.mybir as mybir
    import concourse.tile as tile

    f32 = mybir.dt.float32
    nc = bacc.Bacc("TRN2", target_bir_lowering=False, debug=False)
    wp = nc.dram_tensor("w_pairs", [RPC, 128, 8192], f32, kind="ExternalInput")
    ws = nc.dram_tensor("w_sing", [RPC, 64, 2048], f32, kind="ExternalInput")
    xt = nc.dram_tensor("xt", [RPC + 2, 64, 512], f32, kind="ExternalInput")
    out = nc.dram_tensor("out", [RPC, 4, 16, 8, 64], f32, kind="ExternalOutput")
    wp_ap, ws_ap, xt_ap, out_ap = wp.ap(), ws.ap(), xt.ap(), out.ap()

    with tile.TileContext(nc) as tc:
        with (
            tc.tile_pool(name="wp", bufs=2) as wp_pool,
            tc.tile_pool(name="ws", bufs=2) as ws_pool,
            tc.tile_pool(name="xp", bufs=2) as xp_pool,
            tc.tile_pool(name="stage", bufs=2) as stage_pool,
            tc.tile_pool(name="psum", bufs=8, space="PSUM") as psum_pool,
        ):
            for r in range(RPC):
                wp_t = wp_pool.tile([128, 8192], f32, tag="wp")
                nc.sync.dma_start(wp_t[:], wp_ap[r])
                ws_t = ws_pool.tile([64, 2048], f32, tag="ws")
                nc.sync.dma_start(ws_t[:], ws_ap[r])

                xtiles = []
                for p, (r0, r1, dj0, dj1) in enumerate(PAIRS):
                    xti = xp_pool.tile([128, 576], f32, tag=f"x{p}")
                    # half 0: base placement, valid free [32:544)
                    nc.gpsimd.memset(xti[0:64, 0:32], 0.0)
                    nc.gpsimd.memset(xti[0:64, 544:576], 0.0)
                    nc.sync.dma_start(xti[0:64, 32:544], xt_ap[r + r0])
                    # half 1: shifted by (dj0-dj1) columns
                    lo = 32 + (dj0 - dj1) * 16
                    nc.gpsimd.memset(xti[64:128, 0:lo], 0.0)
                    if lo + 512 < 576:
                        nc.gpsimd.memset(xti[64:128, lo + 512 : 576], 0.0)
                    nc.sync.dma_start(xti[64:128, lo : lo + 512], xt_ap[r + r1])
                    xtiles.append(xti)

                stage = stage_pool.tile([128, 512], f32, tag="stage")
                wp_v = wp_t[:].rearrange("p (o q) -> p o q", q=128)
                ws_v = ws_t[:].rearrange("p (o q) -> p o q", q=32)

                for g in range(8):
                    ps = psum_pool.tile([128, 64], f32, tag="ps")
                    for t in range(5):
                        for jj in range(4):
                            j = 4 * g + jj
                            if t < 4:
                                d = PAIRS[t][2] + 1
                                lhsT = xtiles[t][:, (j + d) * 16 : (j + d + 1) * 16]
                                rhs = wp_v[:, :, 4 * j + t]
                            else:
                                lhsT = xtiles[3][0:64, (j + 3) * 16 : (j + 4) * 16]
                                rhs = ws_v[:, :, j]
                            nc.tensor.matmul(
                                ps[32 * jj : 32 * jj + 16, :],
                                lhsT,
                                rhs,
                                start=(t == 0),
                                stop=(t == 4),
                                tile_position=(0, 32 * jj),
                                skip_group_check=True,
                            )
                    for jj in range(4):
                        nc.vector.tensor_copy(
                            stage[32 * jj : 32 * jj + 16, g * 64 : (g + 1) * 64],
                            ps[32 * jj : 32 * jj + 16, :],
                        )
                for jj in range(4):
                    src = stage[32 * jj : 32 * jj + 16, :].rearrange(
                        "p (g o) -> p g o", o=64
                    )
                    nc.sync.dma_start(out_ap[r][jj], src)
    nc.compile()
    return nc


def _repack_inputs(x, weight):
    x = np.ascontiguousarray(np.asarray(x, dtype=np.float32))
    weight = np.ascontiguousarray(np.asarray(weight, dtype=np.float32))

    wt = np.ascontiguousarray(weight.transpose(2, 1, 0, 3, 4))  # [i, c, o, j, k]
    a = wt[..., :8].reshape(OH, CIN, COUT, OW, 4, 2)  # [i,c,o,j,p,m]
    wp = np.ascontiguousarray(a.transpose(0, 5, 1, 2, 3, 4)).reshape(OH, 128, 8192)
    ws = np.ascontiguousarray(wt[..., 8]).reshape(OH, CIN, 2048)

    xpad = np.zeros((IH + 2, CIN, IW, B), dtype=np.float32)
    xpad[1:33] = x.transpose(1, 3, 2, 0)  # [ih, c, j, b]

    in_maps = []
    for c in range(NCORES):
        in_maps.append(
            {
                "w_pairs": np.ascontiguousarray(wp[c * RPC : (c + 1) * RPC]),
                "w_sing": np.ascontiguousarray(ws[c * RPC : (c + 1) * RPC]),
                "xt": np.ascontiguousarray(
                    xpad[c * RPC : c * RPC + RPC + 2].reshape(RPC + 2, CIN, 512)
                ),
            }
        )
    return in_maps


def _get_nc():
    global _NC
    if _NC is None:
        _NC = _build_nc()
    return _NC


def run_spmd(in_maps, **kwargs):
    from concourse.bass_utils import run_bass_kernel_spmd

    return run_bass_kernel_spmd(
        _get_nc(), in_maps, core_ids=list(range(NCORES)), **kwargs
    )


def kernel(x, weight, bias, _results=None):
    if _results is None:
        _results = run_spmd(_repack_inputs(x, weight)).results
    arr = np.stack([r["out"] for r in _results])  # [core, r, jj, b, g, o]
    out = arr.transpose(3, 0, 1, 4, 2, 5).reshape(B, OH, OW, COUT)
    return out + np.asarray(bias, dtype=np.float32)[None]
